# revision 15
# baseline (speedup 1.0000x reference)
"""LocationAwareAttention Trainium2 kernel (8-core data parallel over batch).

Strategy (per sharding hint): batch is sharded 2-per-core across 8 cores;
k_len stays local so softmax needs no communication.

Per core (2 batches):
  - kproj = key @ w_v done as bf16 matmuls in [d, k]-transposed orientation
    (key pre-transposed host-side per-core so DMA loads are contiguous).
  - loc_energy = tanh(A @ shifted_prev_align + conv_b@w_u) with A = conv_w
    combined with w_u host-side (weights-only transform).
  - e = tanh(kproj + loc_energy + (qproj + bias)) via DVE add + ACT tanh with
    per-partition bias vector.
  - score = score_w . e via matmul with (score_w x onehot_h) stationary,
    accumulating all 8 heads into one [8, 512] PSUM tile.
  - softmax over 4096 keys per (b, h) row on 8/16 partitions.
  - context = align @ v via bf16 matmuls with align transposed on DVE
    (StreamTranspose 32x32 blocks).
"""

import os
from contextlib import ExitStack

import numpy as np
import ml_dtypes

B, KLEN, D_MODEL, HEADS, KERNELS, DIM = 16, 4096, 1024, 8, 10, 128
NCORES = 8
BL = B // NCORES          # batches per core
BH = BL * HEADS           # (b, h) rows per core
KC = 512                  # k chunk (phase 1)
NKC = KLEN // KC          # 8
FC = 128                  # feature chunk
NFC = D_MODEL // FC       # 8

_CACHE: dict = {}


def _build_program():
    import concourse.bass as bass  # noqa: F401
    import concourse.tile as tile
    from concourse import bacc, mybir

    f32, bf16 = mybir.dt.float32, mybir.dt.bfloat16
    AF = mybir.ActivationFunctionType

    nc = bacc.Bacc(
        "TRN2", target_bir_lowering=False, debug=False, num_devices=NCORES
    )

    keyT = nc.dram_tensor("keyT", [BL, D_MODEL, KLEN], bf16, kind="ExternalInput").ap()
    vv = nc.dram_tensor("v", [BL, KLEN, D_MODEL], bf16, kind="ExternalInput").ap()
    pa = nc.dram_tensor("pa", [BL, 24, KLEN], bf16, kind="ExternalInput").ap()
    wv = nc.dram_tensor("wv", [D_MODEL, D_MODEL], bf16, kind="ExternalInput").ap()
    Amat = nc.dram_tensor("A", [24, DIM], bf16, kind="ExternalInput").ap()
    swoh = nc.dram_tensor("swoh", [DIM, 8 * HEADS], bf16, kind="ExternalInput").ap()
    qbT = nc.dram_tensor("qbT", [DIM, BH], f32, kind="ExternalInput").ap()
    cbu = nc.dram_tensor("cbu", [DIM, 1], f32, kind="ExternalInput").ap()
    ctx_out = nc.dram_tensor("ctx", [BH, D_MODEL], f32, kind="ExternalOutput").ap()
    align_out = nc.dram_tensor("align", [BH, KLEN], f32, kind="ExternalOutput").ap()

    with tile.TileContext(nc) as tc, ExitStack() as ctx:
        consts = ctx.enter_context(tc.tile_pool(name="consts", bufs=1))
        pa_pool = ctx.enter_context(tc.tile_pool(name="pa", bufs=1))
        loce_pool = ctx.enter_context(tc.tile_pool(name="loce", bufs=2))
        kt_pool = ctx.enter_context(tc.tile_pool(name="kt", bufs=3))
        epre_pool = ctx.enter_context(tc.tile_pool(name="epre", bufs=4))
        ebf_pool = ctx.enter_context(tc.tile_pool(name="ebf", bufs=5))
        score_pool = ctx.enter_context(tc.tile_pool(name="score", bufs=2))
        smx_pool = ctx.enter_context(tc.tile_pool(name="smx", bufs=1))
        abf_pool = ctx.enter_context(tc.tile_pool(name="abf", bufs=1))
        at_pool = ctx.enter_context(tc.tile_pool(name="at", bufs=2))
        v_pool = ctx.enter_context(tc.tile_pool(name="vp", bufs=3))
        out_pool = ctx.enter_context(tc.tile_pool(name="outp", bufs=2))
        small_pool = ctx.enter_context(tc.tile_pool(name="small", bufs=2))

        kps_pool = ctx.enter_context(tc.tile_pool(name="kpsum", bufs=3, space="PSUM"))
        sps_pool = ctx.enter_context(tc.tile_pool(name="spsum", bufs=2, space="PSUM"))
        lps_pool = ctx.enter_context(tc.tile_pool(name="lpsum", bufs=1, space="PSUM"))
        cps_pool = ctx.enter_context(tc.tile_pool(name="cpsum", bufs=1, space="PSUM"))

        # ---- constants to SBUF ----
        wv_sb = consts.tile([128, NFC * D_MODEL], bf16)  # [128, fc*1024 + e]
        nc.sync.dma_start(
            wv_sb[:].rearrange("p (fc e) -> p fc e", fc=NFC),
            wv.rearrange("(fc p) e -> p fc e", p=128),
        )
        A_sb = consts.tile([24, DIM], bf16)
        nc.sync.dma_start(A_sb[:], Amat)
        swoh_sb = consts.tile([DIM, 8 * HEADS], bf16)
        nc.sync.dma_start(swoh_sb[:], swoh)
        qbT_sb = consts.tile([DIM, BH], f32)
        nc.sync.dma_start(qbT_sb[:], qbT)
        cbu_sb = consts.tile([DIM, 1], f32)
        nc.sync.dma_start(cbu_sb[:], cbu)



        for b in range(BL):
            # ---- location energy: loce[d, k] = tanh(A.T @ pa_shift + cbu) ----
            pa_sb = pa_pool.tile([24, KLEN], bf16)
            nc.sync.dma_start(pa_sb[:], pa[b])
            loce = loce_pool.tile([128, KLEN], bf16)
            for kc in range(NKC):
                lps = lps_pool.tile([128, KC], f32)
                nc.tensor.matmul(
                    lps[:], A_sb[:], pa_sb[:, kc * KC:(kc + 1) * KC],
                    start=True, stop=True,
                )
                nc.scalar.activation(
                    loce[:, kc * KC:(kc + 1) * KC], lps[:], AF.Tanh,
                    bias=cbu_sb[:, 0:1],
                )

            # ---- phase 1: kproj, e, score ----
            score_sb = score_pool.tile([8, KLEN], f32)
            for kc in range(NKC):
                kt = kt_pool.tile([128, NFC * KC], bf16)  # [128, fc*512 + k]
                nc.sync.dma_start(
                    kt[:].rearrange("p (fc k) -> p fc k", fc=NFC),
                    keyT[b].rearrange("(fc p) k -> p fc k", p=128)[
                        :, :, kc * KC:(kc + 1) * KC
                    ],
                )
                sps = sps_pool.tile([8, KC], f32)
                pend = []  # lag score matmuls so PE never waits on DVE/ACT
                for h in range(HEADS):
                    kps = kps_pool.tile([128, KC], f32)
                    for fc in range(NFC):
                        nc.tensor.matmul(
                            kps[:],
                            wv_sb[:, fc * D_MODEL + h * DIM: fc * D_MODEL + (h + 1) * DIM],
                            kt[:, fc * KC:(fc + 1) * KC],
                            start=(fc == 0), stop=(fc == NFC - 1),
                        )
                    epre = epre_pool.tile([128, KC], f32)
                    nc.vector.tensor_add(
                        epre[:], kps[:], loce[:, kc * KC:(kc + 1) * KC]
                    )
                    ebf = ebf_pool.tile([128, KC], bf16)
                    nc.scalar.activation(
                        ebf[:], epre[:], AF.Tanh,
                        bias=qbT_sb[:, b * HEADS + h: b * HEADS + h + 1],
                    )
                    pend.append((h, ebf))
                    if len(pend) > 2:
                        hh, eb = pend.pop(0)
                        nc.tensor.matmul(
                            sps[:], swoh_sb[:, hh * 8:(hh + 1) * 8], eb[:],
                            start=(hh == 0), stop=(hh == HEADS - 1),
                        )
                for hh, eb in pend:
                    nc.tensor.matmul(
                        sps[:], swoh_sb[:, hh * 8:(hh + 1) * 8], eb[:],
                        start=(hh == 0), stop=(hh == HEADS - 1),
                    )
                nc.vector.tensor_copy(score_sb[:, kc * KC:(kc + 1) * KC], sps[:])

            # ---- softmax over k per (b,h) row ----
            mx = small_pool.tile([8, 1], f32, tag="mx")
            nc.vector.reduce_max(mx[:], score_sb[:], axis=mybir.AxisListType.X)
            negmx = small_pool.tile([8, 1], f32, tag="negmx")
            nc.vector.tensor_scalar_mul(negmx[:], mx[:], -1.0)
            p_sb = smx_pool.tile([8, KLEN], f32)
            ssum = small_pool.tile([8, 1], f32, tag="ssum")
            nc.scalar.activation(
                p_sb[:], score_sb[:], AF.Exp, bias=negmx[:, 0:1],
                accum_out=ssum[:],
            )
            rsum = small_pool.tile([8, 1], f32, tag="rsum")
            nc.vector.reciprocal(rsum[:], ssum[:])
            af = smx_pool.tile([8, KLEN], f32, tag="af")
            nc.vector.tensor_scalar_mul(af[:], p_sb[:], rsum[:])
            nc.sync.dma_start(align_out[b * HEADS:(b + 1) * HEADS, :], af[:])

            abf = abf_pool.tile([32, KLEN], bf16)
            nc.vector.memset(abf[:, :], 0.0)
            nc.vector.tensor_scalar_mul(abf[0:8, :], p_sb[:], rsum[:])

            # ---- transpose align to [k, bh] via 32x32 block transposes ----
            atall = at_pool.tile([128, 32 * 32], bf16)  # [128, kc2*32 + row]
            for kc2 in range(32):
                for j in range(4):
                    nc.vector.transpose(
                        atall[32 * j:32 * (j + 1), kc2 * 32:(kc2 + 1) * 32],
                        abf[0:32, kc2 * 128 + 32 * j: kc2 * 128 + 32 * (j + 1)],
                    )

            # ---- context: ctx[h, d] = sum_k align[h, k] v[k, d] ----
            cps0 = cps_pool.tile([8, KC], f32, tag="cps0")
            cps1 = cps_pool.tile([8, KC], f32, tag="cps1")
            for pair in range(16):
                vt = v_pool.tile([128, 2 * D_MODEL], bf16)  # [128, s*1024 + e]
                nc.sync.dma_start(
                    vt[:].rearrange("p (s e) -> p s e", s=2),
                    vv[b, pair * 256:(pair + 1) * 256, :].rearrange(
                        "(s p) e -> p s e", p=128
                    ),
                )
                for s in range(2):
                    kc2 = pair * 2 + s
                    lhs = atall[:, kc2 * 32: kc2 * 32 + 8]
                    for half in range(2):
                        nc.tensor.matmul(
                            (cps0 if half == 0 else cps1)[:],
                            lhs,
                            vt[:, s * D_MODEL + half * 512: s * D_MODEL + (half + 1) * 512],
                            start=(kc2 == 0), stop=(kc2 == 31),
                        )
            ctx_tmp = out_pool.tile([8, D_MODEL], f32)
            nc.vector.tensor_copy(ctx_tmp[:, 0:512], cps0[:])
            nc.vector.tensor_copy(ctx_tmp[:, 512:1024], cps1[:])
            nc.sync.dma_start(ctx_out[b * HEADS:(b + 1) * HEADS, :], ctx_tmp[:])

    nc.compile()
    return nc


def _prep_inputs(query, key_in, value, prev_align, conv_w, conv_b, w_u, w_q,
                 w_v, bias, score_w, score_b):
    bf = ml_dtypes.bfloat16
    # weights-only transforms
    A = np.einsum("cij,cd->ijd", conv_w, w_u).reshape(24, DIM)
    cbu = (conv_b @ w_u).astype(np.float32).reshape(DIM, 1)
    swoh = np.zeros((DIM, 8 * HEADS), np.float32)
    for h in range(HEADS):
        swoh[:, h * 8 + h] = score_w
    wv_bf = np.ascontiguousarray(w_v.astype(bf))
    A_bf = np.ascontiguousarray(A.astype(bf))
    swoh_bf = np.ascontiguousarray(swoh.astype(bf))

    # q projection (small, parameter-sized) + bias folded, [DIM, BH] per core
    qproj = (query[:, 0, :].astype(np.float64) @ w_q.astype(np.float64))
    qproj = qproj.astype(np.float32).reshape(B, HEADS, DIM) + bias[None, None, :]

    # shifted prev_align [B, 24, KLEN]: row (i*3+j) = prev_align[:, i, k+j-1]
    pa = np.zeros((B, HEADS, 3, KLEN), np.float32)
    pa[:, :, 1, :] = prev_align
    pa[:, :, 0, 1:] = prev_align[:, :, :-1]
    pa[:, :, 2, :-1] = prev_align[:, :, 1:]
    pa = pa.reshape(B, 24, KLEN)

    in_maps = []
    for c in range(NCORES):
        bsl = slice(BL * c, BL * (c + 1))
        keyT = np.ascontiguousarray(
            key_in[bsl].transpose(0, 2, 1).astype(bf))        # [BL, 1024, 4096]
        v_bf = np.ascontiguousarray(value[bsl].astype(bf))    # [BL, 4096, 1024]
        pa_bf = np.ascontiguousarray(pa[bsl].astype(bf))      # [BL, 24, 4096]
        qbT = np.ascontiguousarray(
            qproj[bsl].reshape(BH, DIM).T.astype(np.float32))  # [DIM, BH]
        in_maps.append({
            "keyT": keyT, "v": v_bf, "pa": pa_bf, "wv": wv_bf, "A": A_bf,
            "swoh": swoh_bf, "qbT": qbT, "cbu": cbu,
        })
    return in_maps


class _Runner:
    """Compiles the bass program once and keeps inputs staged on-device so
    repeated executions measure device time, not host->device transfer."""

    def __init__(self, nc):
        import jax
        from jax.sharding import Mesh, PartitionSpec, NamedSharding
        from jax.experimental.shard_map import shard_map
        import concourse.mybir as mybir
        from concourse.bass2jax import (
            _bass_exec_p, install_neuronx_cc_hook, partition_id_tensor,
        )

        install_neuronx_cc_hook()
        self.jax = jax

        partition_name = (
            nc.partition_id_tensor.name if nc.partition_id_tensor else None
        )
        in_names, out_names, out_avals = [], [], []
        for alloc in nc.m.functions[0].allocations:
            if not isinstance(alloc, mybir.MemoryLocationSet):
                continue
            name = alloc.memorylocations[0].name
            if alloc.kind == "ExternalInput":
                if name != partition_name:
                    in_names.append(name)
            elif alloc.kind == "ExternalOutput":
                out_names.append(name)
                out_avals.append(jax.core.ShapedArray(
                    tuple(alloc.tensor_shape), mybir.dt.np(alloc.dtype)))
        n_params = len(in_names)
        all_names = in_names + out_names
        if partition_name is not None:
            all_names = all_names + [partition_name]
        donate = tuple(range(n_params, n_params + len(out_names)))

        def _body(*args):
            operands = list(args)
            if partition_name is not None:
                operands.append(partition_id_tensor())
            return tuple(_bass_exec_p.bind(
                *operands,
                out_avals=tuple(out_avals),
                in_names=tuple(all_names),
                out_names=tuple(out_names),
                lowering_input_output_aliases=(),
                sim_require_finite=True,
                sim_require_nnan=True,
                nc=nc,
            ))

        devices = jax.devices()[:NCORES]
        mesh = Mesh(np.asarray(devices), ("core",))
        spec = PartitionSpec("core")
        self.sharding = NamedSharding(mesh, spec)
        self.sharded = jax.jit(
            shard_map(_body, mesh=mesh,
                      in_specs=(spec,) * (n_params + len(out_names)),
                      out_specs=(spec,) * len(out_names), check_rep=False),
            donate_argnums=donate, keep_unused=True,
        )
        self.in_names = in_names
        self.out_names = out_names
        self.out_avals = out_avals
        self.dev_in = None

    def stage(self, in_maps):
        concat_in = [
            np.concatenate([np.asarray(m[name]) for m in in_maps], axis=0)
            for name in self.in_names
        ]
        self.dev_in = [self.jax.device_put(a, self.sharding) for a in concat_in]
        self.jax.block_until_ready(self.dev_in)

    def make_zeros(self):
        zs = [self.jax.device_put(
            np.zeros((NCORES * av.shape[0], *av.shape[1:]), av.dtype),
            self.sharding) for av in self.out_avals]
        self.jax.block_until_ready(zs)
        return zs

    def __call__(self):
        outs = self.sharded(*self.dev_in, *self.make_zeros())
        self.jax.block_until_ready(outs)
        return {n: np.asarray(o) for n, o in zip(self.out_names, outs)}


def _get_runner():
    if "runner" not in _CACHE:
        if "nc" not in _CACHE:
            _CACHE["nc"] = _build_program()
        _CACHE["runner"] = _Runner(_CACHE["nc"])
    return _CACHE["runner"]


def _gather(out):
    ctx_full = out["ctx"]  # [NCORES*BH, D_MODEL]
    ctx = np.stack(
        [ctx_full[r, (r % HEADS) * DIM:((r % HEADS) + 1) * DIM]
         for r in range(B * HEADS)], axis=0,
    ).astype(np.float32)  # [128, 128] diagonal head blocks
    align = out["align"].reshape(B, HEADS, KLEN).astype(np.float32)
    return ctx, align


def kernel(**inputs):
    inputs = {k: np.asarray(v) for k, v in inputs.items()}
    runner = _get_runner()
    runner.stage(_prep_inputs(**inputs))
    return _gather(runner())
